# revision 10
# baseline (speedup 1.0000x reference)
"""Multi-head attention (B=2, S=2048, D=1024, H=16) on 8 trn2 NeuronCores.

Sharding: 2-way batch x 4-way head-group tensor parallel. Core c handles
batch c//4 and heads 4*(c%4) .. 4*(c%4)+3 (a 256-wide feature slice of the
q/k/v projections, and the matching row-slice of the out projection). Each
core emits a full-size [2048, 1024] bf16 partial of the output; the host sums
the 4 partials per batch (f32) and adds the output bias.

v2a data flow (per core):
  - Activations arrive feature-major bf16 ([D, S], host-pretransposed), all
    weights bf16. All matmuls bf16 with f32 PSUM.
  - Q/K feature-major: QT/KT [dq, t] bf16 (no zero-padded K variants). The
    scores matmul for head h is a [64]-contraction bf16 matmul on PE array
    row-half h%2 via an explicit tile_position — even/odd head pairs are
    emitted interleaved so the hardware can pack both 64x128 quadrants.
  - V token-major bf16 with 64 ones-columns appended, so attn.V also yields
    the softmax denominator on psum rows 64..127.
  - exp() on ScalarE reads the scores psum pair [128, 2, 512] and writes PT
    bf16. The Scalar queue carries ONLY the exp stream.
  - Emission is software-pipelined: scores(qc0) ladder with per-chunk K
    projection; V/Q projections and the previous chunk's out-projection are
    interleaved between scores/attnV so PE stays fed while ACT runs exp.
"""

import ml_dtypes
import numpy as np

import concourse.bacc as bacc
import concourse.mybir as mybir
import concourse.tile as tile
from concourse.bass_interp import get_hw_module
from concourse.bass_utils import run_bass_kernel_spmd

# problem constants (hardcoded; must match the reference)
B = 2
S = 2048
D = 1024
NH = 16
DH = 64
SCALE = DH ** -0.5

# sharding
N_CORES = 8
HG = 4                # heads per core
F = HG * DH           # 256 projected features per core
CH = 512              # token chunk
NCH = S // CH         # 4 chunks
P = 128
FT = D // P           # 8 feature tiles
MT = F // P           # 2 projected-feature tiles
KT = S // P           # 16 key-token tiles
KG = KT // 2          # 8 k-tile pairs (psum/exp groups)

f32 = mybir.dt.float32
bf16 = mybir.dt.bfloat16
EXP = mybir.ActivationFunctionType.Exp


def _emit(ctx, nc, tc, aps):
    xqT, xkT, xvT, wqT, wkT, wvT, woT, bq2, bk2, bv1, out = aps

    consts = ctx.enter_context(tc.tile_pool(name="consts", bufs=1))
    persist = ctx.enter_context(tc.tile_pool(name="persist", bufs=1))

    # biases + out-proj weight on the gpsimd DMA queue (scalar queue is
    # reserved for the exp stream; sync queue carries the big x/w stream)
    bq_sb = consts.tile([P, MT], f32)
    bk_sb = consts.tile([P, MT], f32)
    nc.gpsimd.dma_start(out=bq_sb, in_=bq2)
    nc.gpsimd.dma_start(out=bk_sb, in_=bk2)
    bv_sb = consts.tile([P, F], f32)
    nc.gpsimd.dma_start(out=bv_sb, in_=bv1.unsqueeze(0).to_broadcast((P, F)))
    wo_sb = consts.tile([P, MT, D], bf16)
    nc.scalar.dma_start(out=wo_sb, in_=woT)

    # persistent activations
    QT_sb = persist.tile([P, MT, NCH, CH], bf16)   # [dq%128, dq//128, qc, q]
    KT_sb = persist.tile([P, MT, NCH, CH], bf16)
    # V'' layout: [k%128, k//128, h, dv | 64 ones columns]
    V_sb = persist.tile([P, KT, HG, P], bf16)
    nc.gpsimd.memset(V_sb[:, :, :, DH:P], 1.0)

    w_pool = ctx.enter_context(tc.tile_pool(name="w_pool", bufs=2))
    xT_pool = ctx.enter_context(tc.tile_pool(name="xT_pool", bufs=6))
    ps_proj = ctx.enter_context(tc.tile_pool(name="ps_proj", bufs=2, space="PSUM"))
    ps_s = ctx.enter_context(tc.tile_pool(name="ps_s", bufs=2, space="PSUM"))
    ps_av = ctx.enter_context(tc.tile_pool(name="ps_av", bufs=2, space="PSUM"))
    pt_pool = ctx.enter_context(tc.tile_pool(name="pt_pool", bufs=3))
    ot_pool = ctx.enter_context(tc.tile_pool(name="ot_pool", bufs=1))
    o_stage = ctx.enter_context(tc.tile_pool(name="o_stage", bufs=3))
    rc_pool = ctx.enter_context(tc.tile_pool(name="rc_pool", bufs=1))

    OT_sb = ot_pool.tile([P, MT, NCH, CH], bf16)

    # ---- phase-A building blocks -------------------------------------
    def load_w(wT_ap, engine=None):
        w_sb = w_pool.tile([P, FT, F], bf16, tag="w")
        (engine or nc.sync).dma_start(out=w_sb, in_=wT_ap)
        return w_sb

    def load_x(xT_ap, c):
        xT = xT_pool.tile([P, FT, CH], bf16, tag="xT")
        nc.sync.dma_start(out=xT, in_=xT_ap[:, c])
        return xT

    def proj_qk(w_sb, xT, c, is_q):
        b_sb = bq_sb if is_q else bk_sb
        dst = QT_sb if is_q else KT_sb
        for m in range(MT):
            ps = ps_proj.tile([P, CH], f32, tag="proj")
            for ft in range(FT):
                nc.tensor.matmul(
                    ps, w_sb[:, ft, m * P:(m + 1) * P], xT[:, ft, :],
                    start=(ft == 0), stop=(ft == FT - 1),
                )
            nc.vector.tensor_scalar_add(dst[:, m, c, :], ps, b_sb[:, m:m + 1])

    def proj_v(w_sb, xT, c):
        for t4 in range(CH // P):
            ps = ps_proj.tile([P, F], f32, tag="proj")
            for ft in range(FT):
                nc.tensor.matmul(
                    ps, xT[:, ft, t4 * P:(t4 + 1) * P], w_sb[:, ft, :],
                    start=(ft == 0), stop=(ft == FT - 1),
                )
            kt = c * (CH // P) + t4
            nc.vector.tensor_add(
                V_sb[:, kt, :, 0:DH],
                ps.rearrange("p (h d) -> p h d", h=HG),
                bv_sb.rearrange("p (h d) -> p h d", h=HG),
            )

    # ---- phase-B building blocks -------------------------------------
    def new_pt():
        pt = pt_pool.tile([P, KT, CH], bf16, tag="PT")
        return pt

    def scores_pair(qc, mh, PTa, PTb, kg_lo, kg_hi):
        """Interleaved score groups for head pair (2*mh, 2*mh+1).

        Each head's matmul is a [64]-contraction at PE array row-half
        h%2 (tile_position) so the hardware can pack the pair.
        """
        for kg in range(kg_lo, kg_hi):
            psa = ps_s.tile([P, 2, CH], f32, tag="s")
            psb = ps_s.tile([P, 2, CH], f32, tag="s")
            for j in range(2):
                kt = kg * 2 + j
                ktile = (kt // 4, (kt % 4) * P)
                for par, ps in ((0, psa), (1, psb)):
                    p0 = par * DH
                    nc.tensor.matmul(
                        ps[:, j, :],
                        KT_sb[p0:p0 + DH, mh, ktile[0], ktile[1]:ktile[1] + P],
                        QT_sb[p0:p0 + DH, mh, qc, :],
                        start=True, stop=True,
                        tile_position=(p0, 0),
                    )
            nc.scalar.activation(
                out=PTa[:, kg * 2:kg * 2 + 2, :], in_=psa, func=EXP, scale=SCALE
            )
            nc.scalar.activation(
                out=PTb[:, kg * 2:kg * 2 + 2, :], in_=psb, func=EXP, scale=SCALE
            )

    def attnv(qc, h, PT):
        mh, par = divmod(h, 2)
        p0 = par * DH
        po = ps_av.tile([P, CH], f32, tag="o")
        for kt in range(KT):
            nc.tensor.matmul(
                po, V_sb[:, kt, h, :], PT[:, kt, :],
                start=(kt == 0), stop=(kt == KT - 1),
            )
        rs = rc_pool.tile([DH, CH], f32, tag="rs")
        rc = rc_pool.tile([DH, CH], f32, tag="rc")
        nc.vector.tensor_copy(rs, po[DH:P, :])
        nc.vector.reciprocal_approx_fast(rc, rs)
        nc.vector.tensor_mul(OT_sb[p0:p0 + DH, mh, qc, :], po[0:DH, :], rc)

    def outproj(qc):
        for t4 in range(NCH):
            ob = o_stage.tile([P, D], bf16, tag="ob")
            for n2 in range(D // CH):
                ps = ps_av.tile([P, CH], f32, tag="o")
                for m in range(MT):
                    nc.tensor.matmul(
                        ps,
                        OT_sb[:, m, qc, t4 * P:(t4 + 1) * P],
                        wo_sb[:, m, n2 * CH:(n2 + 1) * CH],
                        start=(m == 0), stop=(m == MT - 1),
                    )
                nc.vector.tensor_copy(ob[:, n2 * CH:(n2 + 1) * CH], ps)
            tt = qc * NCH + t4
            nc.gpsimd.dma_start(out=out[tt * P:(tt + 1) * P, :], in_=ob)

    # ---- emission schedule -------------------------------------------
    # lead-in: weight loads on the scalar queue (descriptor-gen in parallel
    # with the x stream on sync; weights are small and arrive early)
    w_k = load_w(wkT, engine=nc.scalar)
    w_q = load_w(wqT, engine=nc.scalar)
    w_v = load_w(wvT, engine=nc.scalar)
    xk = [load_x(xkT, c) for c in range(2)]
    xq0 = load_x(xqT, 0)
    xk += [load_x(xkT, c) for c in range(2, NCH)]

    # ladder: project K per chunk, interleave first head-pair score groups
    pts = {(0, 0): new_pt(), (0, 1): new_pt()}
    for c in range(NCH):
        proj_qk(w_k, xk[c], c, is_q=False)
        if c == 0:
            proj_qk(w_q, xq0, 0, is_q=True)
        scores_pair(0, 0, pts[(0, 0)], pts[(0, 1)], 2 * c, 2 * c + 2)

    # V stream + remaining Q chunks on the DMA queue
    xv = [load_x(xvT, c) for c in range(NCH)]
    xq = {c: load_x(xqT, c) for c in range(1, NCH)}

    def sc_pair(qc, mh):
        pta, ptb = new_pt(), new_pt()
        pts[(qc, 2 * mh)] = pta
        pts[(qc, 2 * mh + 1)] = ptb
        scores_pair(qc, mh, pta, ptb, 0, KG)

    def av(qc, h):
        attnv(qc, h, pts.pop((qc, h)))

    # qc0: V projection fills PE while ACT chews h0/h1 exps
    proj_v(w_v, xv[0], 0)
    proj_v(w_v, xv[1], 1)
    sc_pair(0, 1)
    proj_v(w_v, xv[2], 2)
    proj_v(w_v, xv[3], 3)
    av(0, 0)
    av(0, 1)
    proj_qk(w_q, xq[1], 1, is_q=True)
    av(0, 2)
    av(0, 3)

    for qc in range(1, NCH):
        sc_pair(qc, 0)
        if qc < NCH - 1:
            outproj(qc - 1)
        if qc + 1 < NCH:
            proj_qk(w_q, xq[qc + 1], qc + 1, is_q=True)
        av(qc, 0)
        sc_pair(qc, 1)
        av(qc, 1)
        if qc == NCH - 1:
            outproj(qc - 1)
        av(qc, 2)
        av(qc, 3)
    outproj(NCH - 1)


def _build():
    nc = bacc.Bacc("TRN2", target_bir_lowering=False, debug=False)
    xqT = nc.dram_tensor("xqT", [P, NCH, FT, CH], bf16, kind="ExternalInput").ap()
    xkT = nc.dram_tensor("xkT", [P, NCH, FT, CH], bf16, kind="ExternalInput").ap()
    xvT = nc.dram_tensor("xvT", [P, NCH, FT, CH], bf16, kind="ExternalInput").ap()
    wqT = nc.dram_tensor("wqT", [P, FT, F], bf16, kind="ExternalInput").ap()
    wkT = nc.dram_tensor("wkT", [P, FT, F], bf16, kind="ExternalInput").ap()
    wvT = nc.dram_tensor("wvT", [P, FT, F], bf16, kind="ExternalInput").ap()
    woT = nc.dram_tensor("woT", [P, MT, D], bf16, kind="ExternalInput").ap()
    bq2 = nc.dram_tensor("bq2", [P, MT], f32, kind="ExternalInput").ap()
    bk2 = nc.dram_tensor("bk2", [P, MT], f32, kind="ExternalInput").ap()
    bv1 = nc.dram_tensor("bv1", [F], f32, kind="ExternalInput").ap()
    out = nc.dram_tensor("out", [S, D], bf16, kind="ExternalOutput").ap()
    from contextlib import ExitStack

    with tile.TileContext(nc) as tc, ExitStack() as ctx:
        _emit(ctx, nc, tc,
              (xqT, xkT, xvT, wqT, wkT, wvT, woT, bq2, bk2, bv1, out))
    nc.compile()
    nc.m = get_hw_module(nc.m)
    return nc


_cached_nc = None


def _get_nc():
    global _cached_nc
    if _cached_nc is None:
        _cached_nc = _build()
    return _cached_nc


def make_in_maps(query, key, value, Wq, bq, Wk, bk, Wv, bv, Wo, bo):
    query, key, value, Wq, bq, Wk, bk, Wv, bv, Wo = (
        np.asarray(a, np.float32)
        for a in (query, key, value, Wq, bq, Wk, bk, Wv, bv, Wo)
    )
    bf = ml_dtypes.bfloat16

    def pack_x(x):  # [S, D] -> [P, NCH, FT, CH]
        return np.ascontiguousarray(
            x.reshape(NCH, CH, FT, P).transpose(3, 0, 2, 1)).astype(bf)

    def pack_w(W):  # [F, D] -> [P, FT, F]
        return np.ascontiguousarray(
            W.T.reshape(FT, P, F).transpose(1, 0, 2)).astype(bf)

    xTs = [
        tuple(pack_x(a[b]) for a in (query, key, value))
        for b in range(B)
    ]
    in_maps = []
    for c in range(N_CORES):
        b, g = divmod(c, 4)
        fs = slice(g * F, (g + 1) * F)
        qT, kT, vT = xTs[b]
        in_maps.append({
            "xqT": qT,
            "xkT": kT,
            "xvT": vT,
            "wqT": pack_w(Wq[fs]),
            "wkT": pack_w(Wk[fs]),
            "wvT": pack_w(Wv[fs]),
            "woT": np.ascontiguousarray(
                Wo[:, fs].T.reshape(MT, P, D).transpose(1, 0, 2)).astype(bf),
            "bq2": np.ascontiguousarray(bq[fs].reshape(MT, P).T),
            "bk2": np.ascontiguousarray(bk[fs].reshape(MT, P).T),
            "bv1": np.ascontiguousarray(bv[fs]),
        })
    return in_maps


def combine_outputs(core_outs, bo):
    bo = np.asarray(bo, np.float32)
    out = np.empty((B, S, D), np.float32)
    for b in range(B):
        acc = core_outs[4 * b].astype(np.float32)
        for g in range(1, 4):
            acc = acc + core_outs[4 * b + g].astype(np.float32)
        out[b] = acc + bo
    return out


def kernel(query, key, value, Wq, bq, Wk, bk, Wv, bv, Wo, bo, **run_kwargs):
    nc = _get_nc()
    in_maps = make_in_maps(query, key, value, Wq, bq, Wk, bk, Wv, bv, Wo, bo)
    res = run_bass_kernel_spmd(
        nc, in_maps, core_ids=list(range(N_CORES)), **run_kwargs
    )
    out = combine_outputs([r["out"] for r in res.results], bo)
    if run_kwargs:
        kernel.last_results = res
    return out


# revision 11
# speedup vs baseline: 1.0184x; 1.0184x over previous
"""Multi-head attention (B=2, S=2048, D=1024, H=16) on 8 trn2 NeuronCores.

Sharding: 2-way batch x 4-way head-group tensor parallel. Core c handles
batch c//4 and heads 4*(c%4) .. 4*(c%4)+3 (a 256-wide feature slice of the
q/k/v projections, and the matching row-slice of the out projection). Each
core emits a full-size [2048, 1024] bf16 partial of the output; the host sums
the 4 partials per batch (f32) and adds the output bias.

v2a data flow (per core):
  - Activations arrive feature-major bf16 ([D, S], host-pretransposed), all
    weights bf16. All matmuls bf16 with f32 PSUM.
  - Q/K feature-major: QT/KT [dq, t] bf16 (no zero-padded K variants). The
    scores matmul for head h is a [64]-contraction bf16 matmul on PE array
    row-half h%2 via an explicit tile_position — even/odd head pairs are
    emitted interleaved so the hardware can pack both 64x128 quadrants.
  - V token-major bf16 with 64 ones-columns appended, so attn.V also yields
    the softmax denominator on psum rows 64..127.
  - exp() on ScalarE reads the scores psum pair [128, 2, 512] and writes PT
    bf16. The Scalar queue carries ONLY the exp stream.
  - Emission is software-pipelined: scores(qc0) ladder with per-chunk K
    projection; V/Q projections and the previous chunk's out-projection are
    interleaved between scores/attnV so PE stays fed while ACT runs exp.
"""

import ml_dtypes
import numpy as np

import concourse.bacc as bacc
import concourse.mybir as mybir
import concourse.tile as tile
from concourse.bass_interp import get_hw_module
from concourse.bass_utils import run_bass_kernel_spmd

# problem constants (hardcoded; must match the reference)
B = 2
S = 2048
D = 1024
NH = 16
DH = 64
SCALE = DH ** -0.5

# sharding
N_CORES = 8
HG = 4                # heads per core
F = HG * DH           # 256 projected features per core
CH = 512              # token chunk
NCH = S // CH         # 4 chunks
P = 128
FT = D // P           # 8 feature tiles
MT = F // P           # 2 projected-feature tiles
KT = S // P           # 16 key-token tiles
KG = KT // 2          # 8 k-tile pairs (psum/exp groups)

f32 = mybir.dt.float32
bf16 = mybir.dt.bfloat16
EXP = mybir.ActivationFunctionType.Exp


def _emit(ctx, nc, tc, aps):
    xqT, xkT, xvT, wqT, wkT, wvT, woT, bq2, bk2, bv1, out = aps

    consts = ctx.enter_context(tc.tile_pool(name="consts", bufs=1))
    persist = ctx.enter_context(tc.tile_pool(name="persist", bufs=1))

    # biases + out-proj weight on the gpsimd DMA queue (scalar queue is
    # reserved for the exp stream; sync queue carries the big x/w stream)
    bq_sb = consts.tile([P, MT], f32)
    bk_sb = consts.tile([P, MT], f32)
    nc.gpsimd.dma_start(out=bq_sb, in_=bq2)
    nc.gpsimd.dma_start(out=bk_sb, in_=bk2)
    bv_sb = consts.tile([P, F], f32)
    nc.gpsimd.dma_start(out=bv_sb, in_=bv1.unsqueeze(0).to_broadcast((P, F)))
    wo_sb = consts.tile([P, MT, D], bf16)

    # persistent activations
    QT_sb = persist.tile([P, MT, NCH, CH], bf16)   # [dq%128, dq//128, qc, q]
    KT_sb = persist.tile([P, MT, NCH, CH], bf16)
    # V'' layout: [k%128, k//128, h, dv | 64 ones columns]
    V_sb = persist.tile([P, KT, HG, P], bf16)
    nc.gpsimd.memset(V_sb[:, :, :, DH:P], 1.0)

    w_pool = ctx.enter_context(tc.tile_pool(name="w_pool", bufs=2))
    xT_pool = ctx.enter_context(tc.tile_pool(name="xT_pool", bufs=8))
    ps_proj = ctx.enter_context(tc.tile_pool(name="ps_proj", bufs=2, space="PSUM"))
    ps_s = ctx.enter_context(tc.tile_pool(name="ps_s", bufs=2, space="PSUM"))
    ps_av = ctx.enter_context(tc.tile_pool(name="ps_av", bufs=2, space="PSUM"))
    pt_pool = ctx.enter_context(tc.tile_pool(name="pt_pool", bufs=3))
    ot_pool = ctx.enter_context(tc.tile_pool(name="ot_pool", bufs=1))
    o_stage = ctx.enter_context(tc.tile_pool(name="o_stage", bufs=3))
    rc_pool = ctx.enter_context(tc.tile_pool(name="rc_pool", bufs=1))

    OT_sb = ot_pool.tile([P, MT, NCH, CH], bf16)

    # ---- phase-A building blocks -------------------------------------
    def load_w(wT_ap, engine=None):
        w_sb = w_pool.tile([P, FT, F], bf16, tag="w")
        (engine or nc.sync).dma_start(out=w_sb, in_=wT_ap)
        return w_sb

    def load_x(xT_ap, c):
        xT = xT_pool.tile([P, FT, CH], bf16, tag="xT")
        nc.sync.dma_start(out=xT, in_=xT_ap[:, c])
        return xT

    def proj_qk(w_sb, xT, c, is_q):
        b_sb = bq_sb if is_q else bk_sb
        dst = QT_sb if is_q else KT_sb
        for m in range(MT):
            ps = ps_proj.tile([P, CH], f32, tag="proj")
            for ft in range(FT):
                nc.tensor.matmul(
                    ps, w_sb[:, ft, m * P:(m + 1) * P], xT[:, ft, :],
                    start=(ft == 0), stop=(ft == FT - 1),
                )
            nc.vector.tensor_scalar_add(dst[:, m, c, :], ps, b_sb[:, m:m + 1])

    def proj_v(w_sb, xT, c):
        for t4 in range(CH // P):
            ps = ps_proj.tile([P, F], f32, tag="proj")
            for ft in range(FT):
                nc.tensor.matmul(
                    ps, xT[:, ft, t4 * P:(t4 + 1) * P], w_sb[:, ft, :],
                    start=(ft == 0), stop=(ft == FT - 1),
                )
            kt = c * (CH // P) + t4
            nc.vector.tensor_add(
                V_sb[:, kt, :, 0:DH],
                ps.rearrange("p (h d) -> p h d", h=HG),
                bv_sb.rearrange("p (h d) -> p h d", h=HG),
            )

    # ---- phase-B building blocks -------------------------------------
    def new_pt():
        pt = pt_pool.tile([P, KT, CH], bf16, tag="PT")
        return pt

    def scores_pair(qc, mh, PTa, PTb, kg_lo, kg_hi):
        """Interleaved score groups for head pair (2*mh, 2*mh+1).

        Each head's matmul is a [64]-contraction at PE array row-half
        h%2 (tile_position) so the hardware can pack the pair.
        """
        for kg in range(kg_lo, kg_hi):
            psa = ps_s.tile([P, 2, CH], f32, tag="s")
            psb = ps_s.tile([P, 2, CH], f32, tag="s")
            for j in range(2):
                kt = kg * 2 + j
                ktile = (kt // 4, (kt % 4) * P)
                for par, ps in ((0, psa), (1, psb)):
                    p0 = par * DH
                    nc.tensor.matmul(
                        ps[:, j, :],
                        KT_sb[p0:p0 + DH, mh, ktile[0], ktile[1]:ktile[1] + P],
                        QT_sb[p0:p0 + DH, mh, qc, :],
                        start=True, stop=True,
                        tile_position=(p0, 0),
                    )
            nc.scalar.activation(
                out=PTa[:, kg * 2:kg * 2 + 2, :], in_=psa, func=EXP, scale=SCALE
            )
            nc.scalar.activation(
                out=PTb[:, kg * 2:kg * 2 + 2, :], in_=psb, func=EXP, scale=SCALE
            )

    def attnv(qc, h, PT):
        mh, par = divmod(h, 2)
        p0 = par * DH
        po = ps_av.tile([P, CH], f32, tag="o")
        for kt in range(KT):
            nc.tensor.matmul(
                po, V_sb[:, kt, h, :], PT[:, kt, :],
                start=(kt == 0), stop=(kt == KT - 1),
            )
        rs = rc_pool.tile([DH, CH], f32, tag="rs")
        rc = rc_pool.tile([DH, CH], f32, tag="rc")
        nc.vector.tensor_copy(rs, po[DH:P, :])
        nc.vector.reciprocal_approx_fast(rc, rs)
        nc.vector.tensor_mul(OT_sb[p0:p0 + DH, mh, qc, :], po[0:DH, :], rc)

    def outproj(qc):
        for t4 in range(NCH):
            ob = o_stage.tile([P, D], bf16, tag="ob")
            for n2 in range(D // CH):
                ps = ps_av.tile([P, CH], f32, tag="o")
                for m in range(MT):
                    nc.tensor.matmul(
                        ps,
                        OT_sb[:, m, qc, t4 * P:(t4 + 1) * P],
                        wo_sb[:, m, n2 * CH:(n2 + 1) * CH],
                        start=(m == 0), stop=(m == MT - 1),
                    )
                nc.vector.tensor_copy(ob[:, n2 * CH:(n2 + 1) * CH], ps)
            tt = qc * NCH + t4
            nc.gpsimd.dma_start(out=out[tt * P:(tt + 1) * P, :], in_=ob)

    # ---- emission schedule -------------------------------------------
    # lead-in: one ordered stream on the sync queue == arrival order
    w_k = load_w(wkT)
    xk = [load_x(xkT, c) for c in range(2)]
    w_q = load_w(wqT)
    xq0 = load_x(xqT, 0)
    xk += [load_x(xkT, c) for c in range(2, NCH)]

    # ladder: project K per chunk, interleave first head-pair score groups
    pts = {(0, 0): new_pt(), (0, 1): new_pt()}
    for c in range(NCH):
        proj_qk(w_k, xk[c], c, is_q=False)
        if c == 0:
            proj_qk(w_q, xq0, 0, is_q=True)
        scores_pair(0, 0, pts[(0, 0)], pts[(0, 1)], 2 * c, 2 * c + 2)

    # V stream + wo + remaining Q chunks on the DMA queue
    w_v = load_w(wvT)
    xv = [load_x(xvT, c) for c in range(NCH)]
    nc.sync.dma_start(out=wo_sb, in_=woT)
    xq = {c: load_x(xqT, c) for c in range(1, NCH)}

    def sc_pair(qc, mh):
        pta, ptb = new_pt(), new_pt()
        pts[(qc, 2 * mh)] = pta
        pts[(qc, 2 * mh + 1)] = ptb
        scores_pair(qc, mh, pta, ptb, 0, KG)

    def av(qc, h):
        attnv(qc, h, pts.pop((qc, h)))

    # qc0: V projection fills PE while ACT chews h0/h1 exps
    proj_v(w_v, xv[0], 0)
    proj_v(w_v, xv[1], 1)
    sc_pair(0, 1)
    proj_v(w_v, xv[2], 2)
    proj_v(w_v, xv[3], 3)
    av(0, 0)
    av(0, 1)
    proj_qk(w_q, xq[1], 1, is_q=True)
    av(0, 2)
    av(0, 3)

    for qc in range(1, NCH):
        sc_pair(qc, 0)
        if qc < NCH - 1:
            outproj(qc - 1)
        if qc + 1 < NCH:
            proj_qk(w_q, xq[qc + 1], qc + 1, is_q=True)
        av(qc, 0)
        sc_pair(qc, 1)
        av(qc, 1)
        if qc == NCH - 1:
            outproj(qc - 1)
        av(qc, 2)
        av(qc, 3)
    outproj(NCH - 1)


def _build():
    nc = bacc.Bacc("TRN2", target_bir_lowering=False, debug=False)
    xqT = nc.dram_tensor("xqT", [P, NCH, FT, CH], bf16, kind="ExternalInput").ap()
    xkT = nc.dram_tensor("xkT", [P, NCH, FT, CH], bf16, kind="ExternalInput").ap()
    xvT = nc.dram_tensor("xvT", [P, NCH, FT, CH], bf16, kind="ExternalInput").ap()
    wqT = nc.dram_tensor("wqT", [P, FT, F], bf16, kind="ExternalInput").ap()
    wkT = nc.dram_tensor("wkT", [P, FT, F], bf16, kind="ExternalInput").ap()
    wvT = nc.dram_tensor("wvT", [P, FT, F], bf16, kind="ExternalInput").ap()
    woT = nc.dram_tensor("woT", [P, MT, D], bf16, kind="ExternalInput").ap()
    bq2 = nc.dram_tensor("bq2", [P, MT], f32, kind="ExternalInput").ap()
    bk2 = nc.dram_tensor("bk2", [P, MT], f32, kind="ExternalInput").ap()
    bv1 = nc.dram_tensor("bv1", [F], f32, kind="ExternalInput").ap()
    out = nc.dram_tensor("out", [S, D], bf16, kind="ExternalOutput").ap()
    from contextlib import ExitStack

    with tile.TileContext(nc) as tc, ExitStack() as ctx:
        _emit(ctx, nc, tc,
              (xqT, xkT, xvT, wqT, wkT, wvT, woT, bq2, bk2, bv1, out))
    nc.compile()
    nc.m = get_hw_module(nc.m)
    return nc


_cached_nc = None


def _get_nc():
    global _cached_nc
    if _cached_nc is None:
        _cached_nc = _build()
    return _cached_nc


def make_in_maps(query, key, value, Wq, bq, Wk, bk, Wv, bv, Wo, bo):
    query, key, value, Wq, bq, Wk, bk, Wv, bv, Wo = (
        np.asarray(a, np.float32)
        for a in (query, key, value, Wq, bq, Wk, bk, Wv, bv, Wo)
    )
    bf = ml_dtypes.bfloat16

    def pack_x(x):  # [S, D] -> [P, NCH, FT, CH]
        return np.ascontiguousarray(
            x.reshape(NCH, CH, FT, P).transpose(3, 0, 2, 1)).astype(bf)

    def pack_w(W):  # [F, D] -> [P, FT, F]
        return np.ascontiguousarray(
            W.T.reshape(FT, P, F).transpose(1, 0, 2)).astype(bf)

    xTs = [
        tuple(pack_x(a[b]) for a in (query, key, value))
        for b in range(B)
    ]
    in_maps = []
    for c in range(N_CORES):
        b, g = divmod(c, 4)
        fs = slice(g * F, (g + 1) * F)
        qT, kT, vT = xTs[b]
        in_maps.append({
            "xqT": qT,
            "xkT": kT,
            "xvT": vT,
            "wqT": pack_w(Wq[fs]),
            "wkT": pack_w(Wk[fs]),
            "wvT": pack_w(Wv[fs]),
            "woT": np.ascontiguousarray(
                Wo[:, fs].T.reshape(MT, P, D).transpose(1, 0, 2)).astype(bf),
            "bq2": np.ascontiguousarray(bq[fs].reshape(MT, P).T),
            "bk2": np.ascontiguousarray(bk[fs].reshape(MT, P).T),
            "bv1": np.ascontiguousarray(bv[fs]),
        })
    return in_maps


def combine_outputs(core_outs, bo):
    bo = np.asarray(bo, np.float32)
    out = np.empty((B, S, D), np.float32)
    for b in range(B):
        acc = core_outs[4 * b].astype(np.float32)
        for g in range(1, 4):
            acc = acc + core_outs[4 * b + g].astype(np.float32)
        out[b] = acc + bo
    return out


def kernel(query, key, value, Wq, bq, Wk, bk, Wv, bv, Wo, bo, **run_kwargs):
    nc = _get_nc()
    in_maps = make_in_maps(query, key, value, Wq, bq, Wk, bk, Wv, bv, Wo, bo)
    res = run_bass_kernel_spmd(
        nc, in_maps, core_ids=list(range(N_CORES)), **run_kwargs
    )
    out = combine_outputs([r["out"] for r in res.results], bo)
    if run_kwargs:
        kernel.last_results = res
    return out


# revision 12
# speedup vs baseline: 1.0231x; 1.0046x over previous
"""Multi-head attention (B=2, S=2048, D=1024, H=16) on 8 trn2 NeuronCores.

Sharding: 2-way batch x 4-way head-group tensor parallel. Core c handles
batch c//4 and heads 4*(c%4) .. 4*(c%4)+3 (a 256-wide feature slice of the
q/k/v projections, and the matching row-slice of the out projection). Each
core emits a full-size [2048, 1024] bf16 partial of the output; the host sums
the 4 partials per batch (f32) and adds the output bias.

v2a data flow (per core):
  - Activations arrive feature-major bf16 ([D, S], host-pretransposed), all
    weights bf16. All matmuls bf16 with f32 PSUM.
  - Q/K feature-major: QT/KT [dq, t] bf16 (no zero-padded K variants). The
    scores matmul for head h is a [64]-contraction bf16 matmul on PE array
    row-half h%2 via an explicit tile_position — even/odd head pairs are
    emitted interleaved so the hardware can pack both 64x128 quadrants.
  - V token-major bf16 with 64 ones-columns appended, so attn.V also yields
    the softmax denominator on psum rows 64..127.
  - exp() on ScalarE reads the scores psum pair [128, 2, 512] and writes PT
    bf16. The Scalar queue carries ONLY the exp stream.
  - Emission is software-pipelined: scores(qc0) ladder with per-chunk K
    projection; V/Q projections and the previous chunk's out-projection are
    interleaved between scores/attnV so PE stays fed while ACT runs exp.
"""

import ml_dtypes
import numpy as np

import concourse.bacc as bacc
import concourse.mybir as mybir
import concourse.tile as tile
from concourse.bass_interp import get_hw_module
from concourse.bass_utils import run_bass_kernel_spmd

# problem constants (hardcoded; must match the reference)
B = 2
S = 2048
D = 1024
NH = 16
DH = 64
SCALE = DH ** -0.5

# sharding
N_CORES = 8
HG = 4                # heads per core
F = HG * DH           # 256 projected features per core
CH = 512              # token chunk
NCH = S // CH         # 4 chunks
P = 128
FT = D // P           # 8 feature tiles
MT = F // P           # 2 projected-feature tiles
KT = S // P           # 16 key-token tiles
KG = KT // 2          # 8 k-tile pairs (psum/exp groups)

f32 = mybir.dt.float32
bf16 = mybir.dt.bfloat16
EXP = mybir.ActivationFunctionType.Exp


def _emit(ctx, nc, tc, aps):
    xqT, xkT, xvT, wqT, wkT, wvT, woT, bq2, bk2, bv1, out = aps

    consts = ctx.enter_context(tc.tile_pool(name="consts", bufs=1))
    persist = ctx.enter_context(tc.tile_pool(name="persist", bufs=1))

    # biases + out-proj weight on the gpsimd DMA queue (scalar queue is
    # reserved for the exp stream; sync queue carries the big x/w stream)
    bq_sb = consts.tile([P, MT], f32)
    bk_sb = consts.tile([P, MT], f32)
    nc.gpsimd.dma_start(out=bq_sb, in_=bq2)
    nc.gpsimd.dma_start(out=bk_sb, in_=bk2)
    bv_sb = consts.tile([P, F], f32)
    nc.gpsimd.dma_start(out=bv_sb, in_=bv1.unsqueeze(0).to_broadcast((P, F)))
    wo_sb = consts.tile([P, MT, D], bf16)

    # persistent activations
    QT_sb = persist.tile([P, MT, NCH, CH], bf16)   # [dq%128, dq//128, qc, q]
    KT_sb = persist.tile([P, MT, NCH, CH], bf16)
    # V'' layout: [k%128, k//128, h, dv | 64 ones columns]
    V_sb = persist.tile([P, KT, HG, P], bf16)
    nc.gpsimd.memset(V_sb[:, :, :, DH:P], 1.0)

    w_pool = ctx.enter_context(tc.tile_pool(name="w_pool", bufs=2))
    xT_pool = ctx.enter_context(tc.tile_pool(name="xT_pool", bufs=8))
    ps_proj = ctx.enter_context(tc.tile_pool(name="ps_proj", bufs=2, space="PSUM"))
    ps_s = ctx.enter_context(tc.tile_pool(name="ps_s", bufs=2, space="PSUM"))
    ps_av = ctx.enter_context(tc.tile_pool(name="ps_av", bufs=2, space="PSUM"))
    pt_pool = ctx.enter_context(tc.tile_pool(name="pt_pool", bufs=3))
    ot_pool = ctx.enter_context(tc.tile_pool(name="ot_pool", bufs=1))
    o_stage = ctx.enter_context(tc.tile_pool(name="o_stage", bufs=3))
    rc_pool = ctx.enter_context(tc.tile_pool(name="rc_pool", bufs=1))

    OT_sb = ot_pool.tile([P, MT, NCH, CH], bf16)

    # PE warmup: chained dummy matmuls spin the PE through the initial DMA
    # wait so the clock is at full p-state when real work arrives.
    warm_sb = consts.tile([P, CH], bf16)
    nc.vector.memset(warm_sb, 0.0)
    ps_warm = ps_proj.tile([P, CH], f32, tag="proj")
    for i in range(48):
        nc.tensor.matmul(
            ps_warm, warm_sb[:, 0:P], warm_sb,
            start=(i == 0), stop=(i == 47),
        )

    # ---- phase-A building blocks -------------------------------------
    def load_w(wT_ap, engine=None):
        w_sb = w_pool.tile([P, FT, F], bf16, tag="w")
        (engine or nc.sync).dma_start(out=w_sb, in_=wT_ap)
        return w_sb

    def load_x(xT_ap, c):
        xT = xT_pool.tile([P, FT, CH], bf16, tag="xT")
        nc.sync.dma_start(out=xT, in_=xT_ap[:, c])
        return xT

    def proj_qk(w_sb, xT, c, is_q):
        b_sb = bq_sb if is_q else bk_sb
        dst = QT_sb if is_q else KT_sb
        for m in range(MT):
            ps = ps_proj.tile([P, CH], f32, tag="proj")
            for ft in range(FT):
                nc.tensor.matmul(
                    ps, w_sb[:, ft, m * P:(m + 1) * P], xT[:, ft, :],
                    start=(ft == 0), stop=(ft == FT - 1),
                )
            nc.vector.tensor_scalar_add(dst[:, m, c, :], ps, b_sb[:, m:m + 1])

    def proj_v(w_sb, xT, c):
        for t4 in range(CH // P):
            ps = ps_proj.tile([P, F], f32, tag="proj")
            for ft in range(FT):
                nc.tensor.matmul(
                    ps, xT[:, ft, t4 * P:(t4 + 1) * P], w_sb[:, ft, :],
                    start=(ft == 0), stop=(ft == FT - 1),
                )
            kt = c * (CH // P) + t4
            nc.vector.tensor_add(
                V_sb[:, kt, :, 0:DH],
                ps.rearrange("p (h d) -> p h d", h=HG),
                bv_sb.rearrange("p (h d) -> p h d", h=HG),
            )

    # ---- phase-B building blocks -------------------------------------
    def new_pt():
        pt = pt_pool.tile([P, KT, CH], bf16, tag="PT")
        return pt

    def scores_pair(qc, mh, PTa, PTb, kg_lo, kg_hi):
        """Interleaved score groups for head pair (2*mh, 2*mh+1).

        Each head's matmul is a [64]-contraction at PE array row-half
        h%2 (tile_position) so the hardware can pack the pair.
        """
        for kg in range(kg_lo, kg_hi):
            psa = ps_s.tile([P, 2, CH], f32, tag="s")
            psb = ps_s.tile([P, 2, CH], f32, tag="s")
            for j in range(2):
                kt = kg * 2 + j
                ktile = (kt // 4, (kt % 4) * P)
                for par, ps in ((0, psa), (1, psb)):
                    p0 = par * DH
                    nc.tensor.matmul(
                        ps[:, j, :],
                        KT_sb[p0:p0 + DH, mh, ktile[0], ktile[1]:ktile[1] + P],
                        QT_sb[p0:p0 + DH, mh, qc, :],
                        start=True, stop=True,
                        tile_position=(p0, 0),
                    )
            nc.scalar.activation(
                out=PTa[:, kg * 2:kg * 2 + 2, :], in_=psa, func=EXP, scale=SCALE
            )
            nc.scalar.activation(
                out=PTb[:, kg * 2:kg * 2 + 2, :], in_=psb, func=EXP, scale=SCALE
            )

    def attnv(qc, h, PT):
        mh, par = divmod(h, 2)
        p0 = par * DH
        po = ps_av.tile([P, CH], f32, tag="o")
        for kt in range(KT):
            nc.tensor.matmul(
                po, V_sb[:, kt, h, :], PT[:, kt, :],
                start=(kt == 0), stop=(kt == KT - 1),
            )
        rs = rc_pool.tile([DH, CH], f32, tag="rs")
        rc = rc_pool.tile([DH, CH], f32, tag="rc")
        nc.vector.tensor_copy(rs, po[DH:P, :])
        nc.vector.reciprocal_approx_fast(rc, rs)
        nc.vector.tensor_mul(OT_sb[p0:p0 + DH, mh, qc, :], po[0:DH, :], rc)

    def outproj(qc):
        for t4 in range(NCH):
            ob = o_stage.tile([P, D], bf16, tag="ob")
            for n2 in range(D // CH):
                ps = ps_av.tile([P, CH], f32, tag="o")
                for m in range(MT):
                    nc.tensor.matmul(
                        ps,
                        OT_sb[:, m, qc, t4 * P:(t4 + 1) * P],
                        wo_sb[:, m, n2 * CH:(n2 + 1) * CH],
                        start=(m == 0), stop=(m == MT - 1),
                    )
                nc.vector.tensor_copy(ob[:, n2 * CH:(n2 + 1) * CH], ps)
            tt = qc * NCH + t4
            nc.gpsimd.dma_start(out=out[tt * P:(tt + 1) * P, :], in_=ob)

    # ---- emission schedule -------------------------------------------
    # lead-in: one ordered stream on the sync queue == arrival order
    w_k = load_w(wkT)
    xk = [load_x(xkT, c) for c in range(2)]
    w_q = load_w(wqT)
    xq0 = load_x(xqT, 0)
    xk += [load_x(xkT, c) for c in range(2, NCH)]

    # ladder: project K per chunk, interleave first head-pair score groups
    pts = {(0, 0): new_pt(), (0, 1): new_pt()}
    for c in range(NCH):
        proj_qk(w_k, xk[c], c, is_q=False)
        if c == 0:
            proj_qk(w_q, xq0, 0, is_q=True)
        scores_pair(0, 0, pts[(0, 0)], pts[(0, 1)], 2 * c, 2 * c + 2)

    # V stream + wo + remaining Q chunks on the DMA queue
    w_v = load_w(wvT)
    xv = [load_x(xvT, c) for c in range(NCH)]
    nc.sync.dma_start(out=wo_sb, in_=woT)
    xq = {c: load_x(xqT, c) for c in range(1, NCH)}

    def sc_pair(qc, mh):
        pta, ptb = new_pt(), new_pt()
        pts[(qc, 2 * mh)] = pta
        pts[(qc, 2 * mh + 1)] = ptb
        scores_pair(qc, mh, pta, ptb, 0, KG)

    def av(qc, h):
        attnv(qc, h, pts.pop((qc, h)))

    # qc0: V projection fills PE while ACT chews h0/h1 exps
    proj_v(w_v, xv[0], 0)
    proj_v(w_v, xv[1], 1)
    sc_pair(0, 1)
    proj_v(w_v, xv[2], 2)
    proj_v(w_v, xv[3], 3)
    av(0, 0)
    av(0, 1)
    proj_qk(w_q, xq[1], 1, is_q=True)
    av(0, 2)
    av(0, 3)

    for qc in range(1, NCH):
        sc_pair(qc, 0)
        if qc < NCH - 1:
            outproj(qc - 1)
        if qc + 1 < NCH:
            proj_qk(w_q, xq[qc + 1], qc + 1, is_q=True)
        av(qc, 0)
        sc_pair(qc, 1)
        av(qc, 1)
        if qc == NCH - 1:
            outproj(qc - 1)
        av(qc, 2)
        av(qc, 3)
    outproj(NCH - 1)


def _build():
    nc = bacc.Bacc("TRN2", target_bir_lowering=False, debug=False)
    xqT = nc.dram_tensor("xqT", [P, NCH, FT, CH], bf16, kind="ExternalInput").ap()
    xkT = nc.dram_tensor("xkT", [P, NCH, FT, CH], bf16, kind="ExternalInput").ap()
    xvT = nc.dram_tensor("xvT", [P, NCH, FT, CH], bf16, kind="ExternalInput").ap()
    wqT = nc.dram_tensor("wqT", [P, FT, F], bf16, kind="ExternalInput").ap()
    wkT = nc.dram_tensor("wkT", [P, FT, F], bf16, kind="ExternalInput").ap()
    wvT = nc.dram_tensor("wvT", [P, FT, F], bf16, kind="ExternalInput").ap()
    woT = nc.dram_tensor("woT", [P, MT, D], bf16, kind="ExternalInput").ap()
    bq2 = nc.dram_tensor("bq2", [P, MT], f32, kind="ExternalInput").ap()
    bk2 = nc.dram_tensor("bk2", [P, MT], f32, kind="ExternalInput").ap()
    bv1 = nc.dram_tensor("bv1", [F], f32, kind="ExternalInput").ap()
    out = nc.dram_tensor("out", [S, D], bf16, kind="ExternalOutput").ap()
    from contextlib import ExitStack

    with tile.TileContext(nc) as tc, ExitStack() as ctx:
        _emit(ctx, nc, tc,
              (xqT, xkT, xvT, wqT, wkT, wvT, woT, bq2, bk2, bv1, out))
    nc.compile()
    nc.m = get_hw_module(nc.m)
    return nc


_cached_nc = None


def _get_nc():
    global _cached_nc
    if _cached_nc is None:
        _cached_nc = _build()
    return _cached_nc


def make_in_maps(query, key, value, Wq, bq, Wk, bk, Wv, bv, Wo, bo):
    query, key, value, Wq, bq, Wk, bk, Wv, bv, Wo = (
        np.asarray(a, np.float32)
        for a in (query, key, value, Wq, bq, Wk, bk, Wv, bv, Wo)
    )
    bf = ml_dtypes.bfloat16

    def pack_x(x):  # [S, D] -> [P, NCH, FT, CH]
        return np.ascontiguousarray(
            x.reshape(NCH, CH, FT, P).transpose(3, 0, 2, 1)).astype(bf)

    def pack_w(W):  # [F, D] -> [P, FT, F]
        return np.ascontiguousarray(
            W.T.reshape(FT, P, F).transpose(1, 0, 2)).astype(bf)

    xTs = [
        tuple(pack_x(a[b]) for a in (query, key, value))
        for b in range(B)
    ]
    in_maps = []
    for c in range(N_CORES):
        b, g = divmod(c, 4)
        fs = slice(g * F, (g + 1) * F)
        qT, kT, vT = xTs[b]
        in_maps.append({
            "xqT": qT,
            "xkT": kT,
            "xvT": vT,
            "wqT": pack_w(Wq[fs]),
            "wkT": pack_w(Wk[fs]),
            "wvT": pack_w(Wv[fs]),
            "woT": np.ascontiguousarray(
                Wo[:, fs].T.reshape(MT, P, D).transpose(1, 0, 2)).astype(bf),
            "bq2": np.ascontiguousarray(bq[fs].reshape(MT, P).T),
            "bk2": np.ascontiguousarray(bk[fs].reshape(MT, P).T),
            "bv1": np.ascontiguousarray(bv[fs]),
        })
    return in_maps


def combine_outputs(core_outs, bo):
    bo = np.asarray(bo, np.float32)
    out = np.empty((B, S, D), np.float32)
    for b in range(B):
        acc = core_outs[4 * b].astype(np.float32)
        for g in range(1, 4):
            acc = acc + core_outs[4 * b + g].astype(np.float32)
        out[b] = acc + bo
    return out


def kernel(query, key, value, Wq, bq, Wk, bk, Wv, bv, Wo, bo, **run_kwargs):
    nc = _get_nc()
    in_maps = make_in_maps(query, key, value, Wq, bq, Wk, bk, Wv, bv, Wo, bo)
    res = run_bass_kernel_spmd(
        nc, in_maps, core_ids=list(range(N_CORES)), **run_kwargs
    )
    out = combine_outputs([r["out"] for r in res.results], bo)
    if run_kwargs:
        kernel.last_results = res
    return out
